# revision 1
# baseline (speedup 1.0000x reference)
"""Trainium2 Bass kernel for nn_ApplyTimeChannel.

y[b,r,c,m] = sum_{a,l} h_time[b,r,c,0,a,m,l] * xp[b,0,a,g[m,l]]
with B=32, RX=1, RXA=16, TX=1, TXA=4, NT=2048, L=16, T=2063.

Strategy (data-parallel over batch, 4 batches per core, no collectives):
  host: gather xg = xp[..., g]  (tiny vs h), pre-transpose h and xg so that
        SBUF partition p = (mh, a, l) with mh = which half of the padded
        2064-sample output-time axis, and the free dim is mq (1032).
  dev:  per (b, c): DVE computes prod[p, mq] = h*xg (f32 in, bf16 out);
        PE contracts the 64-wide (a,l) axis per half using a constant
        ones-block stationary whose column (2c+mh) routes each c's result
        into PSUM rows 2c:2c+2 of a shared [32, 512] accumulation bank
        (start on c==0);  ACT evicts PSUM -> SBUF;  DMA out.
"""

import sys

if "/opt/trn_rl_repo" not in sys.path:
    sys.path.insert(0, "/opt/trn_rl_repo")

import numpy as np

B, C, A, NT, L, T = 32, 16, 4, 2048, 16, 2063
MH, MQ = 2, 1032  # padded T = 2064 = MH * MQ
P = 128  # partitions = MH * A * L
NCORES = 8
BS = B // NCORES  # batches per core
NBLK = ((0, 512), (512, 512), (1024, 8))  # mq -> psum bank blocks
CBLK = 4  # c's per h DMA (2.1 MB transfers)
HBUFS = 5
PBUFS = 8

TRACE = False
LAST = {}

_CACHE = {}


def _build_nc():
    import concourse.bacc as bacc
    import concourse.mybir as mybir
    import concourse.tile as tile

    f32 = mybir.dt.float32
    bf16 = mybir.dt.bfloat16

    nc = bacc.Bacc("TRN2", target_bir_lowering=False, debug=False)
    hh = nc.dram_tensor("hh", [BS, P, C, MQ], f32, kind="ExternalInput")
    vv = nc.dram_tensor("vv", [BS, P, MQ], bf16, kind="ExternalInput")
    ww = nc.dram_tensor("ww", [P, C * 32], bf16, kind="ExternalInput")
    out = nc.dram_tensor("out", [BS, 2 * C, MQ], f32, kind="ExternalOutput")

    from concourse.tile import add_dep_helper

    with tile.TileContext(nc) as tc:
        with (
            tc.tile_pool(name="wpool", bufs=1) as wpool,
            tc.tile_pool(name="vpool", bufs=BS) as vpool,
            tc.tile_pool(name="hpool", bufs=HBUFS) as hpool,
            tc.tile_pool(name="ppool", bufs=PBUFS) as ppool,
            tc.tile_pool(name="ypool", bufs=2) as ypool,
            tc.tile_pool(name="pspool", bufs=6, space="PSUM") as pspool,
        ):
            # The h stream owns the SWDGE (gpsimd) queue; w rides the scalar
            # HWDGE ring and the v tiles the sync HWDGE ring so neither ever
            # queues behind h traffic.
            wb = wpool.tile([P, C * 32], bf16)
            nc.scalar.dma_start(out=wb[:], in_=ww[:])
            # ~4.5us of dummy matmuls on scratch data during the DMA-boot
            # window: trips the PE HAM clock-gate to 2.4 GHz before the
            # real matmuls arrive. Results land in a spare psum bank and
            # are never read.
            wsc = wpool.tile([P, 32], bf16, tag="wsc")
            nc.vector.memset(wsc[:], 0)
            xsc = wpool.tile([P, 512], bf16, tag="xsc")
            nc.vector.memset(xsc[:], 0)
            pssc = pspool.tile([32, 512], f32, tag="pssc", bufs=1)
            warm_prev = None
            for i in range(18):
                wmm = nc.tensor.matmul(
                    out=pssc[:], lhsT=wsc[:], rhs=xsc[:], start=True, stop=True
                )
                if warm_prev is not None:
                    add_dep_helper(wmm.ins, warm_prev, sync=False,
                                   reason="warmup chain")
                warm_prev = wmm.ins
            vts = []
            for b in range(BS):
                vt = vpool.tile([P, MQ], bf16, tag="v", name=f"v{b}")
                nc.sync.dma_start(out=vt[:], in_=vv[b])
                vts.append(vt)

            # c-block sizes per batch: fine-grained tail on the last batch so
            # the exposed compute after the final h DMA stays small.
            def cblocks(b):
                if b == BS - 1:
                    return [4, 4, 4, 2, 1, 1]
                return [CBLK] * (C // CBLK)

            for b in range(BS):
                psums = [
                    pspool.tile([2 * C, n], f32, tag="psum", name=f"ps{b}_{i}")
                    for i, (_, n) in enumerate(NBLK)
                ]

                def mms(pt, c, lo, hi):
                    for blk, (off, n) in enumerate(NBLK):
                        if off >= hi or off + n <= lo:
                            continue
                        nc.tensor.matmul(
                            out=psums[blk][:, :],
                            lhsT=wb[:, c * 32 : (c + 1) * 32],
                            rhs=pt[:, off : off + n],
                            start=(c == 0),
                            stop=(c == C - 1),
                        )

                c0 = 0
                for bi, nb in enumerate(cblocks(b)):
                    ht = hpool.tile([P, CBLK, MQ], bf16, tag="ht")
                    nc.gpsimd.dma_start(
                        out=ht[:, :nb, :], in_=hh[b, :, c0 : c0 + nb, :]
                    )
                    for cc in range(nb):
                        c = c0 + cc
                        pt = ppool.tile([P, MQ], bf16)
                        nc.vector.tensor_mul(out=pt[:], in0=ht[:, cc, :], in1=vts[b][:])
                        mms(pt, c, 0, MQ)
                    c0 += nb
                if b < BS - 1:
                    yt = ypool.tile([2 * C, MQ], f32)
                    for blk, (off, n) in enumerate(NBLK):
                        # parallel eviction: ACT takes banks 0/2, DVE bank 1
                        eng = nc.vector if blk == 1 else nc.scalar
                        if eng is nc.vector:
                            eng.tensor_copy(
                                out=yt[:, off : off + n], in_=psums[blk][:, :]
                            )
                        else:
                            eng.copy(out=yt[:, off : off + n], in_=psums[blk][:, :])
                    nc.scalar.dma_start(out=out[b], in_=yt[:])
                else:
                    # last batch: separate tiles per psum bank so each
                    # evict -> store -> HBM-receipt pipeline runs
                    # independently; runt first (it gates the kernel end)
                    y2 = ypool.tile([2 * C, 8], f32, tag="y2")
                    nc.scalar.copy(out=y2[:], in_=psums[2][:, :])
                    nc.sync.dma_start(out=out[b, :, 1024:MQ], in_=y2[:])
                    y0 = ypool.tile([2 * C, 512], f32, tag="y0")
                    nc.scalar.copy(out=y0[:], in_=psums[0][:, :])
                    nc.sync.dma_start(out=out[b, :, 0:512], in_=y0[:])
                    y1 = ypool.tile([2 * C, 512], f32, tag="y1")
                    nc.vector.tensor_copy(out=y1[:], in_=psums[1][:, :])
                    nc.scalar.dma_start(out=out[b, :, 512:1024], in_=y1[:])

    nc.compile()
    return nc


def _get_nc():
    if "nc" not in _CACHE:
        _CACHE["nc"] = _build_nc()
    return _CACHE["nc"]


def _make_ww():
    import ml_dtypes
    ww = np.zeros((P, C * 32), np.float32)
    for c in range(C):
        for mh in range(MH):
            ww[mh * 64 : (mh + 1) * 64, c * 32 + 2 * c + mh] = 1.0
    return ww.astype(ml_dtypes.bfloat16)


def _prep_inputs(x, h_time, g):
    x = np.asarray(x, dtype=np.float32)
    h = np.asarray(h_time, dtype=np.float32)
    g = np.asarray(g)

    # host gather: xg[b, a, m, l] = xp[b, a, g[m, l]]
    xsq = x.reshape(B, A, NT)
    xp = np.zeros((B, A, NT + 1), np.float32)
    xp[:, :, :NT] = xsq
    gi = np.clip(g.astype(np.int64), 0, NT)
    xg = xp[:, :, gi]  # [B, A, T, L]

    xgp = np.zeros((B, A, MH * MQ, L), np.float32)
    xgp[:, :, :T] = xg
    import ml_dtypes
    vv = xgp.reshape(B, A, MH, MQ, L).transpose(0, 2, 1, 4, 3).reshape(B, P, MQ)
    vv = np.ascontiguousarray(vv).astype(ml_dtypes.bfloat16)

    hsq = h.reshape(B, C, A, T, L)
    hp = np.zeros((B, C, A, MH * MQ, L), np.float32)
    hp[:, :, :, :T] = hsq
    hh = (
        hp.reshape(B, C, A, MH, MQ, L)
        .transpose(0, 3, 2, 5, 1, 4)
        .reshape(B, P, C, MQ)
    )
    hh = np.ascontiguousarray(hh)
    return hh, vv, _make_ww()


def _postprocess(res_list):
    # per-core out: [BS, 2C, MQ] with row r = 2c + mh
    y = np.concatenate([np.asarray(r["out"]) for r in res_list], axis=0)
    y = y.reshape(B, C, MH, MQ).reshape(B, C, MH * MQ)[:, :, :T]
    return np.ascontiguousarray(y.reshape(B, 1, C, T).astype(np.float32))


def kernel(x, h_time, g):
    from concourse.bass_utils import run_bass_kernel_spmd

    hh, vv, ww = _prep_inputs(x, h_time, g)
    in_maps = []
    for i in range(NCORES):
        sl = slice(i * BS, (i + 1) * BS)
        in_maps.append({"hh": hh[sl], "vv": vv[sl], "ww": ww})

    nc = _get_nc()
    kw = {}
    if TRACE and LAST.get("trace_cores"):
        kw["trace_cores"] = LAST["trace_cores"]
    res = run_bass_kernel_spmd(
        nc, in_maps, core_ids=list(range(NCORES)), trace=TRACE, **kw
    )
    LAST["exec_time_ns"] = res.exec_time_ns
    LAST["result"] = res
    return _postprocess(res.results)

